# revision 11
# baseline (speedup 1.0000x reference)
"""Distributed causal-attention block (dense_transformer) on 8 TRN2 NeuronCores.

Sharding: data-parallel over batch (b=2) x tensor-parallel over head pairs
(8 heads -> 4 groups of 2). Core i handles batch i//4, heads (2*(i%4), 2*(i%4)+1).
Per-core: QKV projection for its 2 heads (transposed layouts so attention is
transpose-free), block-causal flash-style attention (S^T = K @ Q^T formulation,
softmax denominator via an augmented ones-column in V), partial output
projection, then ReduceScatter(add) over each 4-core batch group.

B, S, D, H = 2, 4096, 512, 8 (hd=64). Hardcoded per problem spec.
"""

import numpy as np
import ml_dtypes

import concourse.bass as bass
import concourse.bacc as bacc
import concourse.mybir as mybir
from concourse import tile
from concourse.bass_utils import run_bass_kernel_spmd

B, S, D = 2, 4096, 512
H = 8
HD = D // H          # 64
NCORES = 8
R = 128              # qkv rows per core (2 heads x 64)
S4 = S // 4          # reduce-scatter shard rows
NT = 8               # q tiles of 512
QW = 512             # q tile width

BF16 = mybir.dt.bfloat16
F32 = mybir.dt.float32
AF = mybir.ActivationFunctionType
BF16_NP = ml_dtypes.bfloat16

_CACHE = {}


def _build_nc():
    nc = bacc.Bacc(num_devices=NCORES)

    xT = nc.declare_dram_parameter("xT", [D, S], BF16, isOutput=False)
    wqT = nc.declare_dram_parameter("wqT", [D, R], BF16, isOutput=False)
    wkT = nc.declare_dram_parameter("wkT", [D, R], BF16, isOutput=False)
    wvT = nc.declare_dram_parameter("wvT", [D, 130], BF16, isOutput=False)
    bq = nc.declare_dram_parameter("bq", [R, 1], F32, isOutput=False)
    bk = nc.declare_dram_parameter("bk", [R, 1], F32, isOutput=False)
    bvb = nc.declare_dram_parameter("bvb", [128, 130], F32, isOutput=False)
    wo0 = nc.declare_dram_parameter("wo0", [HD, D], BF16, isOutput=False)
    wo1 = nc.declare_dram_parameter("wo1", [HD, D], BF16, isOutput=False)
    bob4 = nc.declare_dram_parameter("bob4", [128, D], F32, isOutput=False)
    maskc = nc.declare_dram_parameter("maskc", [128, 4 * QW], BF16, isOutput=False)
    out_ext = nc.declare_dram_parameter("out", [S4, D], F32, isOutput=True)

    parts = [nc.dram_tensor(f"part{t}", [QW, D], BF16) for t in range(NT)]
    rss = [nc.dram_tensor(f"rs{t}", [QW // 4, D], BF16) for t in range(NT)]

    with tile.TileContext(nc) as tc:
        with (
            tc.tile_pool(name="const", bufs=1) as cpool,
            tc.tile_pool(name="xres", bufs=1) as xpool,
            tc.tile_pool(name="pt", bufs=6) as ppool,
            tc.tile_pool(name="small", bufs=2) as spool,
            tc.tile_pool(name="stage", bufs=3) as stpool,
            tc.tile_pool(name="ps_s", bufs=2, space="PSUM") as ps_s,
            tc.tile_pool(name="ps_o", bufs=2, space="PSUM") as ps_o,
        ):
            # ---------- constants / weights into SBUF ----------
            wq_sb = cpool.tile([128, D], BF16)
            nc.sync.dma_start(wq_sb[:].rearrange("p (c m) -> p c m", c=4),
                              wqT[:, :].rearrange("(c p) m -> p c m", p=128))
            wk_sb = cpool.tile([128, D], BF16)
            nc.sync.dma_start(wk_sb[:].rearrange("p (c m) -> p c m", c=4),
                              wkT[:, :].rearrange("(c p) m -> p c m", p=128))
            wv_sb = cpool.tile([128, 4 * 130], BF16)
            nc.sync.dma_start(wv_sb[:].rearrange("p (c m) -> p c m", c=4),
                              wvT[:, :].rearrange("(c p) m -> p c m", p=128))
            wo0_sb = cpool.tile([HD, D], BF16)
            nc.sync.dma_start(wo0_sb[:], wo0[:, :])
            wo1_sb = cpool.tile([HD, D], BF16)
            nc.sync.dma_start(wo1_sb[:], wo1[:, :])
            bq_sb = cpool.tile([R, 1], F32)
            nc.sync.dma_start(bq_sb[:], bq[:, :])
            bk_sb = cpool.tile([R, 1], F32)
            nc.sync.dma_start(bk_sb[:], bk[:, :])
            bvb_sb = cpool.tile([128, 130], F32)
            nc.sync.dma_start(bvb_sb[:], bvb[:, :])
            bob4_sb = cpool.tile([128, D], F32)
            nc.sync.dma_start(bob4_sb[:], bob4[:, :])
            mask_sb = cpool.tile([128, 4 * QW], BF16)
            nc.sync.dma_start(mask_sb[:], maskc[:, :])
            ones_sb = cpool.tile([128, HD], F32)
            nc.vector.memset(ones_sb[:], 1.0)

            xt = []
            for c in range(4):
                t_ = xpool.tile([128, S], BF16, tag=f"xt{c}")
                nc.sync.dma_start(t_[:], xT[128 * c:128 * (c + 1), :])
                xt.append(t_)

            # ---------- Q^T / K^T projections: [128 rows, S] ----------
            qT = xpool.tile([128, S], BF16, tag="qT")
            kT = xpool.tile([128, S], BF16, tag="kT")
            for w_sb, b_sb, dst in ((wq_sb, bq_sb, qT), (wk_sb, bk_sb, kT)):
                for nt in range(NT):
                    ps = ps_s.tile([128, QW], F32, tag="s")
                    for c in range(4):
                        nc.tensor.matmul(
                            ps[:],
                            w_sb[:, 128 * c:128 * (c + 1)],
                            xt[c][:, QW * nt:QW * (nt + 1)],
                            start=(c == 0), stop=(c == 3),
                        )
                    nc.vector.tensor_scalar_add(
                        dst[:, QW * nt:QW * (nt + 1)], ps[:], b_sb[:])

            # ---------- V natural (augmented with ones col per head) ----------
            # vaug[:, 130*tb : 130*tb+130] = [V_h0 | 1 | V_h1 | 1] for token block tb
            vaug = xpool.tile([128, 32 * 130], BF16, tag="vaug")
            for tb in range(32):
                ps = ps_s.tile([128, QW], F32, tag="s")
                for c in range(4):
                    nc.tensor.matmul(
                        ps[:, 0:130],
                        xt[c][:, 128 * tb:128 * (tb + 1)],
                        wv_sb[:, 130 * c:130 * (c + 1)],
                        start=(c == 0), stop=(c == 3),
                    )
                nc.vector.tensor_add(
                    vaug[:, 130 * tb:130 * (tb + 1)], ps[:, 0:130], bvb_sb[:]
                )

            # ---------- attention over q tiles ----------
            for t in range(NT):
                nj = 4 * t + 4          # causal: k blocks 0 .. 4t+3
                o0 = ps_o.tile([65, QW], F32, tag="o0")
                o1 = ps_o.tile([65, QW], F32, tag="o1")
                for j in range(nj):
                    # causal: q columns < q0 are fully masked for this k block
                    q0 = max(0, 128 * (j - 4 * t))
                    live = QW - q0
                    s = ps_s.tile([128, 2 * QW], F32, tag="s")
                    for h in (0, 1):
                        nc.tensor.matmul(
                            s[:, QW * h + q0:QW * (h + 1)],
                            kT[64 * h:64 * (h + 1), 128 * j:128 * (j + 1)],
                            qT[64 * h:64 * (h + 1), QW * t + q0:QW * (t + 1)],
                            start=True, stop=True,
                        )
                    p = ppool.tile([128, 2 * QW], BF16, tag="p")
                    if q0 == 0:
                        nc.scalar.activation(p[:], s[:], AF.Exp, bias=0.0, scale=0.125)
                    else:
                        sv = s[:].rearrange("k (h q) -> k h q", h=2)[:, :, q0:QW]
                        pv = p[:].rearrange("k (h q) -> k h q", h=2)[:, :, q0:QW]
                        nc.scalar.activation(pv, sv, AF.Exp, bias=0.0, scale=0.125)
                    if j >= 4 * t:  # diagonal 128-col boundary: 0/1 mask (r=0 tile)
                        for h in (0, 1):
                            nc.vector.tensor_mul(
                                p[:, QW * h + q0:QW * h + q0 + 128],
                                p[:, QW * h + q0:QW * h + q0 + 128],
                                mask_sb[:, 0:128],
                            )
                    for h, oo in ((0, o0), (1, o1)):
                        nc.tensor.matmul(
                            oo[:, q0:QW],
                            vaug[:, 130 * j + 65 * h:130 * j + 65 * (h + 1)],
                            p[:, QW * h + q0:QW * (h + 1)],
                            start=(j == 0), stop=(j == nj - 1),
                        )

                # ---- softmax denominators (row 64 of o0/o1), normalize, O-proj
                # broadcast l (row 64 of o0/o1) across 64 partitions via a
                # Kc=1 matmul, then reciprocal on the base-0 tile
                # (reciprocal_approx_fast misbehaves on nonzero base partitions)
                lrow = spool.tile([128, 2 * QW], F32, tag="lrow")
                nc.vector.tensor_copy(lrow[64:65, 0:QW], o0[64:65, :])
                nc.vector.tensor_copy(lrow[64:65, QW:2 * QW], o1[64:65, :])

                bcs = []
                for h in range(2):
                    bc = ps_s.tile([128, QW], F32, tag="s")
                    nc.tensor.matmul(
                        bc[0:64, :],
                        ones_sb[64:65, :],
                        lrow[64:65, QW * h:QW * (h + 1)],
                        start=True, stop=True,
                    )
                    bcsb = spool.tile([HD, QW], F32, tag=f"bcsb{h}")
                    nc.vector.tensor_copy(bcsb[:], bc[0:64, :])
                    bcsi = spool.tile([HD, QW], F32, tag=f"bcsi{h}")
                    nc.vector.reciprocal_approx_fast(bcsi[:], bcsb[:])
                    bcs.append(bcsi)

                otn0 = spool.tile([HD, QW], BF16, tag="otn0")
                otn1 = spool.tile([HD, QW], BF16, tag="otn1")
                nc.vector.tensor_mul(otn0[:], o0[0:64, :], bcs[0][:])
                nc.vector.tensor_mul(otn1[:], o1[0:64, :], bcs[1][:])

                for tb in range(4):
                    po = ps_s.tile([128, QW], F32, tag="s")
                    nc.tensor.matmul(po[:], otn0[:, 128 * tb:128 * (tb + 1)],
                                     wo0_sb[:], start=True, stop=False)
                    nc.tensor.matmul(po[:], otn1[:, 128 * tb:128 * (tb + 1)],
                                     wo1_sb[:], start=False, stop=True)
                    st = stpool.tile([128, QW], BF16, tag="st")
                    nc.vector.tensor_add(st[:], po[:], bob4_sb[:])
                    nc.sync.dma_start(
                        parts[t][128 * tb:128 * (tb + 1), :], st[:]
                    )
                # chunked ReduceScatter: overlaps with later q tiles
                nc.gpsimd.collective_compute(
                    "ReduceScatter",
                    mybir.AluOpType.add,
                    replica_groups=[[0, 1, 2, 3], [4, 5, 6, 7]],
                    ins=[parts[t][:, :]],
                    outs=[rss[t][:, :]],
                )

            # ---------- gather chunks, cast to f32, write output ----------
            for t in range(NT):
                g = stpool.tile([128, D], BF16, tag="g")
                nc.sync.dma_start(g[:], rss[t][:, :])
                gf = stpool.tile([128, D], F32, tag="gf")
                nc.vector.tensor_copy(gf[:], g[:])
                nc.sync.dma_start(out_ext[128 * t:128 * (t + 1), :], gf[:])

    nc.finalize()
    return nc


def _make_in_maps(x, Wqkv, bqkv, Wo, bo):
    # causal 0/1 multiplicative masks for the 4 diagonal sub-block offsets:
    # keep (p, o) where o >= 128*r + p  (k = 128*(4t+r)+p, q = 512*t+o)
    p_idx = np.arange(128)[:, None]
    o_idx = np.arange(QW)[None, :]
    maskc = np.concatenate(
        [(o_idx >= 128 * r + p_idx).astype(np.float32) for r in range(4)], axis=1
    ).astype(BF16_NP)

    in_maps = []
    for core in range(NCORES):
        b = core // 4
        g = core % 4
        rows = slice(128 * g, 128 * (g + 1))
        wq = Wqkv[0:D][rows]            # [128, 512]
        wk = Wqkv[D:2 * D][rows]
        wv = Wqkv[2 * D:3 * D][rows]
        wvT = np.zeros((D, 130), dtype=np.float32)
        wvT[:, 0:64] = wv[0:64].T
        wvT[:, 65:129] = wv[64:128].T
        bvb = np.zeros((128, 130), dtype=np.float32)
        bvb[:, 0:64] = bqkv[2 * D:3 * D][rows][0:64][None, :]
        bvb[:, 64] = 1.0
        bvb[:, 65:129] = bqkv[2 * D:3 * D][rows][64:128][None, :]
        bvb[:, 129] = 1.0
        in_maps.append({
            "xT": np.ascontiguousarray(x[b].T).astype(BF16_NP),
            "wqT": np.ascontiguousarray(wq.T).astype(BF16_NP),
            "wkT": np.ascontiguousarray(wk.T).astype(BF16_NP),
            "wvT": wvT.astype(BF16_NP),
            "bq": np.ascontiguousarray(bqkv[0:D][rows][:, None]).astype(np.float32),
            "bk": np.ascontiguousarray(bqkv[D:2 * D][rows][:, None]).astype(np.float32),
            "bvb": bvb,
            "wo0": np.ascontiguousarray(Wo[:, 128 * g:128 * g + 64].T).astype(BF16_NP),
            "wo1": np.ascontiguousarray(Wo[:, 128 * g + 64:128 * (g + 1)].T).astype(BF16_NP),
            "bob4": np.tile((bo / 4.0).astype(np.float32)[None, :], (128, 1)),
            "maskc": maskc,
        })
    return in_maps


def run(x, Wqkv, bqkv, Wo, bo, trace=False):
    if "nc" not in _CACHE:
        _CACHE["nc"] = _build_nc()
    nc = _CACHE["nc"]
    in_maps = _make_in_maps(x, Wqkv, bqkv, Wo, bo)
    res = run_bass_kernel_spmd(nc, in_maps, core_ids=list(range(NCORES)), trace=trace)
    out = np.empty((B, S, D), dtype=np.float32)
    for core in range(NCORES):
        b = core // 4
        r = core % 4
        o = res.results[core]["out"]
        # chunked ReduceScatter: rank r holds rows 512t+128r..+128 per q tile t
        for t in range(NT):
            out[b, QW * t + 128 * r:QW * t + 128 * (r + 1), :] = o[128 * t:128 * (t + 1)]
    return out, res


def kernel(x, Wqkv, bqkv, Wo, bo):
    out, _ = run(np.asarray(x, dtype=np.float32), np.asarray(Wqkv, dtype=np.float32),
                 np.asarray(bqkv, dtype=np.float32), np.asarray(Wo, dtype=np.float32),
                 np.asarray(bo, dtype=np.float32))
    return out


# revision 14
# speedup vs baseline: 1.0463x; 1.0463x over previous
"""Distributed causal-attention block (dense_transformer) on 8 TRN2 NeuronCores.

Sharding: data-parallel over batch (b=2) x tensor-parallel over head pairs
(8 heads -> 4 groups of 2). Core i handles batch i//4, heads (2*(i%4), 2*(i%4)+1).
Per-core: QKV projection for its 2 heads (transposed layouts so attention is
transpose-free), block-causal flash-style attention (S^T = K @ Q^T formulation,
softmax denominator via an augmented ones-column in V), partial output
projection, then ReduceScatter(add) over each 4-core batch group.

B, S, D, H = 2, 4096, 512, 8 (hd=64). Hardcoded per problem spec.
"""

import numpy as np
import ml_dtypes

import concourse.bass as bass
import concourse.bacc as bacc
import concourse.mybir as mybir
from concourse import tile
from concourse.bass_utils import run_bass_kernel_spmd

B, S, D = 2, 4096, 512
H = 8
HD = D // H          # 64
NCORES = 8
R = 128              # qkv rows per core (2 heads x 64)
S4 = S // 4          # reduce-scatter shard rows
NT = 8               # q tiles of 512
QW = 512             # q tile width

BF16 = mybir.dt.bfloat16
F32 = mybir.dt.float32
AF = mybir.ActivationFunctionType
BF16_NP = ml_dtypes.bfloat16

_CACHE = {}


def _build_nc():
    nc = bacc.Bacc(num_devices=NCORES)

    xT = nc.declare_dram_parameter("xT", [D, S], BF16, isOutput=False)
    wqT = nc.declare_dram_parameter("wqT", [D, R], BF16, isOutput=False)
    wkT = nc.declare_dram_parameter("wkT", [D, R], BF16, isOutput=False)
    wvT = nc.declare_dram_parameter("wvT", [D, 130], BF16, isOutput=False)
    bq = nc.declare_dram_parameter("bq", [R, 1], F32, isOutput=False)
    bk = nc.declare_dram_parameter("bk", [R, 1], F32, isOutput=False)
    bvb = nc.declare_dram_parameter("bvb", [128, 130], F32, isOutput=False)
    wo0 = nc.declare_dram_parameter("wo0", [HD, D], BF16, isOutput=False)
    wo1 = nc.declare_dram_parameter("wo1", [HD, D], BF16, isOutput=False)
    bob4 = nc.declare_dram_parameter("bob4", [128, D], F32, isOutput=False)
    maskc = nc.declare_dram_parameter("maskc", [128, 4 * QW], BF16, isOutput=False)
    out_ext = nc.declare_dram_parameter("out", [S4, D], F32, isOutput=True)

    parts = [nc.dram_tensor(f"part{t}", [QW, D], BF16) for t in range(NT)]
    ldram = [nc.dram_tensor(f"ldram{t}", [2 * QW], F32) for t in range(NT)]
    rss = [nc.dram_tensor(f"rs{t}", [QW // 4, D], BF16) for t in range(NT)]

    with tile.TileContext(nc) as tc:
        with (
            tc.tile_pool(name="const", bufs=1) as cpool,
            tc.tile_pool(name="xres", bufs=1) as xpool,
            tc.tile_pool(name="pt", bufs=6) as ppool,
            tc.tile_pool(name="small", bufs=2) as spool,
            tc.tile_pool(name="stage", bufs=3) as stpool,
            tc.tile_pool(name="ps_s", bufs=3, space="PSUM") as ps_s,
            tc.tile_pool(name="ps_o", bufs=1, space="PSUM") as ps_o,
        ):
            # ---------- constants / weights into SBUF ----------
            wq_sb = cpool.tile([128, D], BF16)
            nc.sync.dma_start(wq_sb[:].rearrange("p (c m) -> p c m", c=4),
                              wqT[:, :].rearrange("(c p) m -> p c m", p=128))
            wk_sb = cpool.tile([128, D], BF16)
            nc.sync.dma_start(wk_sb[:].rearrange("p (c m) -> p c m", c=4),
                              wkT[:, :].rearrange("(c p) m -> p c m", p=128))
            wv_sb = cpool.tile([128, 4 * 130], BF16)
            nc.sync.dma_start(wv_sb[:].rearrange("p (c m) -> p c m", c=4),
                              wvT[:, :].rearrange("(c p) m -> p c m", p=128))
            wo0_sb = cpool.tile([HD, D], BF16)
            nc.sync.dma_start(wo0_sb[:], wo0[:, :])
            wo1_sb = cpool.tile([HD, D], BF16)
            nc.sync.dma_start(wo1_sb[:], wo1[:, :])
            bq_sb = cpool.tile([R, 1], F32)
            nc.sync.dma_start(bq_sb[:], bq[:, :])
            bk_sb = cpool.tile([R, 1], F32)
            nc.sync.dma_start(bk_sb[:], bk[:, :])
            bvb_sb = cpool.tile([128, 130], F32)
            nc.sync.dma_start(bvb_sb[:], bvb[:, :])
            bob4_sb = cpool.tile([128, D], F32)
            nc.sync.dma_start(bob4_sb[:], bob4[:, :])
            mask_sb = cpool.tile([128, 4 * QW], BF16)
            nc.sync.dma_start(mask_sb[:], maskc[:, :])
            ones_sb = cpool.tile([128, HD], F32)
            nc.vector.memset(ones_sb[:], 1.0)

            xt = []
            for c in range(4):
                t_ = xpool.tile([128, S], BF16, tag=f"xt{c}")
                nc.sync.dma_start(t_[:], xT[128 * c:128 * (c + 1), :])
                xt.append(t_)

            # ---------- Q^T / K^T projections: [128 rows, S] ----------
            qT = xpool.tile([128, S], BF16, tag="qT")
            kT = xpool.tile([128, S], BF16, tag="kT")
            for w_sb, b_sb, dst in ((wq_sb, bq_sb, qT), (wk_sb, bk_sb, kT)):
                for nt in range(NT):
                    ps = ps_s.tile([128, QW], F32, tag="s")
                    for c in range(4):
                        nc.tensor.matmul(
                            ps[:],
                            w_sb[:, 128 * c:128 * (c + 1)],
                            xt[c][:, QW * nt:QW * (nt + 1)],
                            start=(c == 0), stop=(c == 3),
                        )
                    nc.vector.tensor_scalar_add(
                        dst[:, QW * nt:QW * (nt + 1)], ps[:], b_sb[:])

            # ---------- V natural (augmented with ones col per head) ----------
            # vaug[:, 130*tb : 130*tb+130] = [V_h0 | 1 | V_h1 | 1] for token block tb
            vaug = xpool.tile([128, 32 * 130], BF16, tag="vaug")
            for tb in range(32):
                ps = ps_s.tile([128, QW], F32, tag="s")
                for c in range(4):
                    nc.tensor.matmul(
                        ps[:, 0:130],
                        xt[c][:, 128 * tb:128 * (tb + 1)],
                        wv_sb[:, 130 * c:130 * (c + 1)],
                        start=(c == 0), stop=(c == 3),
                    )
                nc.vector.tensor_add(
                    vaug[:, 130 * tb:130 * (tb + 1)], ps[:, 0:130], bvb_sb[:]
                )

            # ---------- attention over q tiles ----------
            for t in range(NT):
                nj = 4 * t + 4          # causal: k blocks 0 .. 4t+3
                o0 = ps_o.tile([65, QW], F32, tag="o0")
                o1 = ps_o.tile([65, QW], F32, tag="o1")
                for j in range(nj):
                    # causal: q columns < q0 are fully masked for this k block
                    q0 = max(0, 128 * (j - 4 * t))
                    live = QW - q0
                    s = ps_s.tile([128, 2 * QW], F32, tag="s")
                    for h in (0, 1):
                        nc.tensor.matmul(
                            s[:, QW * h + q0:QW * (h + 1)],
                            kT[64 * h:64 * (h + 1), 128 * j:128 * (j + 1)],
                            qT[64 * h:64 * (h + 1), QW * t + q0:QW * (t + 1)],
                            start=True, stop=True,
                        )
                    p = ppool.tile([128, 2 * QW], BF16, tag="p")
                    if q0 == 0:
                        nc.scalar.activation(p[:], s[:], AF.Exp, bias=0.0, scale=0.125)
                    else:
                        sv = s[:].rearrange("k (h q) -> k h q", h=2)[:, :, q0:QW]
                        pv = p[:].rearrange("k (h q) -> k h q", h=2)[:, :, q0:QW]
                        nc.scalar.activation(pv, sv, AF.Exp, bias=0.0, scale=0.125)
                    if j >= 4 * t:  # diagonal 128-col boundary: 0/1 mask (r=0 tile)
                        for h in (0, 1):
                            nc.vector.tensor_mul(
                                p[:, QW * h + q0:QW * h + q0 + 128],
                                p[:, QW * h + q0:QW * h + q0 + 128],
                                mask_sb[:, 0:128],
                            )
                    for h, oo in ((0, o0), (1, o1)):
                        nc.tensor.matmul(
                            oo[:, q0:QW],
                            vaug[:, 130 * j + 65 * h:130 * j + 65 * (h + 1)],
                            p[:, QW * h + q0:QW * (h + 1)],
                            start=(j == 0), stop=(j == nj - 1),
                        )

                # ---- softmax denominators (row 64 of o0/o1), normalize, O-proj
                # free the o accumulators fast: copy O^T (rows 0..63) to SBUF
                # and the l rows (row 64) to an SBUF staging row; the O-proj
                # only depends on the copies, not on the reciprocal path.
                oc0 = spool.tile([HD, QW], BF16, tag="oc0")
                oc1 = spool.tile([HD, QW], BF16, tag="oc1")
                nc.vector.tensor_copy(oc0[:], o0[0:64, :])
                nc.vector.tensor_copy(oc1[:], o1[0:64, :])
                lrow = spool.tile([128, 2 * QW], F32, tag="lrow")
                nc.vector.tensor_copy(lrow[64:65, 0:QW], o0[64:65, :])
                nc.vector.tensor_copy(lrow[64:65, QW:2 * QW], o1[64:65, :])

                # l -> natural per-token-partition layout [128, tb] via DMA,
                # then reciprocal (base-0 only: reciprocal_approx_fast quirk)
                l_nat = spool.tile([128, 8], F32, tag="l_nat")
                nc.sync.dma_start(ldram[t][:].rearrange("(a q) -> a q", a=1),
                                  lrow[64:65, :])
                for h in range(2):
                    nc.sync.dma_start(
                        l_nat[:, 4 * h:4 * (h + 1)],
                        ldram[t][QW * h:QW * (h + 1)].rearrange(
                            "(tb p) -> p tb", p=128),
                    )
                linv_nat = spool.tile([128, 8], F32, tag="linv_nat")
                nc.vector.reciprocal_approx_fast(linv_nat[:], l_nat[:])

                for tb in range(4):
                    po0 = ps_s.tile([128, QW], F32, tag="s")
                    po1 = ps_s.tile([128, QW], F32, tag="s")
                    nc.tensor.matmul(po0[:], oc0[:, 128 * tb:128 * (tb + 1)],
                                     wo0_sb[:], start=True, stop=True)
                    nc.tensor.matmul(po1[:], oc1[:, 128 * tb:128 * (tb + 1)],
                                     wo1_sb[:], start=True, stop=True)
                    tmp = stpool.tile([128, QW], F32, tag="tmp")
                    nc.vector.scalar_tensor_tensor(
                        tmp[:], po0[:], linv_nat[:, tb:tb + 1], bob4_sb[:],
                        mybir.AluOpType.mult, mybir.AluOpType.add)
                    st = stpool.tile([128, QW], BF16, tag="st")
                    nc.vector.scalar_tensor_tensor(
                        st[:], po1[:], linv_nat[:, 4 + tb:5 + tb], tmp[:],
                        mybir.AluOpType.mult, mybir.AluOpType.add)
                    nc.sync.dma_start(
                        parts[t][128 * tb:128 * (tb + 1), :], st[:]
                    )
                # chunked ReduceScatter: overlaps with later q tiles
                nc.gpsimd.collective_compute(
                    "ReduceScatter",
                    mybir.AluOpType.add,
                    replica_groups=[[0, 1, 2, 3], [4, 5, 6, 7]],
                    ins=[parts[t][:, :]],
                    outs=[rss[t][:, :]],
                )

            # ---------- gather chunks, cast to f32, write output ----------
            for t in range(NT):
                g = stpool.tile([128, D], BF16, tag="g")
                nc.sync.dma_start(g[:], rss[t][:, :])
                gf = stpool.tile([128, D], F32, tag="gf")
                nc.vector.tensor_copy(gf[:], g[:])
                nc.sync.dma_start(out_ext[128 * t:128 * (t + 1), :], gf[:])

    nc.finalize()
    return nc


def _make_in_maps(x, Wqkv, bqkv, Wo, bo):
    # causal 0/1 multiplicative masks for the 4 diagonal sub-block offsets:
    # keep (p, o) where o >= 128*r + p  (k = 128*(4t+r)+p, q = 512*t+o)
    p_idx = np.arange(128)[:, None]
    o_idx = np.arange(QW)[None, :]
    maskc = np.concatenate(
        [(o_idx >= 128 * r + p_idx).astype(np.float32) for r in range(4)], axis=1
    ).astype(BF16_NP)

    in_maps = []
    for core in range(NCORES):
        b = core // 4
        g = core % 4
        rows = slice(128 * g, 128 * (g + 1))
        wq = Wqkv[0:D][rows]            # [128, 512]
        wk = Wqkv[D:2 * D][rows]
        wv = Wqkv[2 * D:3 * D][rows]
        wvT = np.zeros((D, 130), dtype=np.float32)
        wvT[:, 0:64] = wv[0:64].T
        wvT[:, 65:129] = wv[64:128].T
        bvb = np.zeros((128, 130), dtype=np.float32)
        bvb[:, 0:64] = bqkv[2 * D:3 * D][rows][0:64][None, :]
        bvb[:, 64] = 1.0
        bvb[:, 65:129] = bqkv[2 * D:3 * D][rows][64:128][None, :]
        bvb[:, 129] = 1.0
        in_maps.append({
            "xT": np.ascontiguousarray(x[b].T).astype(BF16_NP),
            "wqT": np.ascontiguousarray(wq.T).astype(BF16_NP),
            "wkT": np.ascontiguousarray(wk.T).astype(BF16_NP),
            "wvT": wvT.astype(BF16_NP),
            "bq": np.ascontiguousarray(bqkv[0:D][rows][:, None]).astype(np.float32),
            "bk": np.ascontiguousarray(bqkv[D:2 * D][rows][:, None]).astype(np.float32),
            "bvb": bvb,
            "wo0": np.ascontiguousarray(Wo[:, 128 * g:128 * g + 64].T).astype(BF16_NP),
            "wo1": np.ascontiguousarray(Wo[:, 128 * g + 64:128 * (g + 1)].T).astype(BF16_NP),
            "bob4": np.tile((bo / 4.0).astype(np.float32)[None, :], (128, 1)),
            "maskc": maskc,
        })
    return in_maps


def run(x, Wqkv, bqkv, Wo, bo, trace=False):
    if "nc" not in _CACHE:
        _CACHE["nc"] = _build_nc()
    nc = _CACHE["nc"]
    in_maps = _make_in_maps(x, Wqkv, bqkv, Wo, bo)
    res = run_bass_kernel_spmd(nc, in_maps, core_ids=list(range(NCORES)), trace=trace)
    out = np.empty((B, S, D), dtype=np.float32)
    for core in range(NCORES):
        b = core // 4
        r = core % 4
        o = res.results[core]["out"]
        # chunked ReduceScatter: rank r holds rows 512t+128r..+128 per q tile t
        for t in range(NT):
            out[b, QW * t + 128 * r:QW * t + 128 * (r + 1), :] = o[128 * t:128 * (t + 1)]
    return out, res


def kernel(x, Wqkv, bqkv, Wo, bo):
    out, _ = run(np.asarray(x, dtype=np.float32), np.asarray(Wqkv, dtype=np.float32),
                 np.asarray(bqkv, dtype=np.float32), np.asarray(Wo, dtype=np.float32),
                 np.asarray(bo, dtype=np.float32))
    return out


# revision 15
# speedup vs baseline: 1.2158x; 1.1620x over previous
"""Distributed causal-attention block (dense_transformer) on 8 TRN2 NeuronCores.

Sharding: data-parallel over batch (b=2) x tensor-parallel over head pairs
(8 heads -> 4 groups of 2). Core i handles batch i//4, heads (2*(i%4), 2*(i%4)+1).
Per-core: QKV projection for its 2 heads (transposed layouts so attention is
transpose-free), block-causal flash-style attention (S^T = K @ Q^T formulation,
softmax denominator via an augmented ones-column in V), partial output
projection, then ReduceScatter(add) over each 4-core batch group.

B, S, D, H = 2, 4096, 512, 8 (hd=64). Hardcoded per problem spec.
"""

import numpy as np
import ml_dtypes

import concourse.bass as bass
import concourse.bacc as bacc
import concourse.mybir as mybir
from concourse import tile
from concourse.bass_utils import run_bass_kernel_spmd

B, S, D = 2, 4096, 512
H = 8
HD = D // H          # 64
NCORES = 8
R = 128              # qkv rows per core (2 heads x 64)
S4 = S // 4          # reduce-scatter shard rows
NT = 8               # q tiles of 512
QW = 512             # q tile width

BF16 = mybir.dt.bfloat16
F32 = mybir.dt.float32
AF = mybir.ActivationFunctionType
BF16_NP = ml_dtypes.bfloat16

_CACHE = {}


def _build_nc():
    nc = bacc.Bacc(num_devices=NCORES)

    xT = nc.declare_dram_parameter("xT", [D, S], BF16, isOutput=False)
    wqT = nc.declare_dram_parameter("wqT", [D, R], BF16, isOutput=False)
    wkT = nc.declare_dram_parameter("wkT", [D, R], BF16, isOutput=False)
    wvT = nc.declare_dram_parameter("wvT", [D, 130], BF16, isOutput=False)
    bq = nc.declare_dram_parameter("bq", [R, 1], F32, isOutput=False)
    bk = nc.declare_dram_parameter("bk", [R, 1], F32, isOutput=False)
    bvb = nc.declare_dram_parameter("bvb", [128, 130], F32, isOutput=False)
    wo0 = nc.declare_dram_parameter("wo0", [HD, D], BF16, isOutput=False)
    wo1 = nc.declare_dram_parameter("wo1", [HD, D], BF16, isOutput=False)
    bob4 = nc.declare_dram_parameter("bob4", [128, D], F32, isOutput=False)
    maskc = nc.declare_dram_parameter("maskc", [128, 4 * QW], BF16, isOutput=False)
    out_ext = nc.declare_dram_parameter("out", [S4, D], F32, isOutput=True)

    parts = [nc.dram_tensor(f"part{t}", [QW, D], BF16) for t in range(NT)]
    ldram = [nc.dram_tensor(f"ldram{t}", [2 * QW], F32) for t in range(NT)]
    rss = [nc.dram_tensor(f"rs{t}", [QW // 4, D], BF16) for t in range(NT)]

    with tile.TileContext(nc) as tc:
        with (
            tc.tile_pool(name="const", bufs=1) as cpool,
            tc.tile_pool(name="xres", bufs=1) as xpool,
            tc.tile_pool(name="pt", bufs=6) as ppool,
            tc.tile_pool(name="small", bufs=2) as spool,
            tc.tile_pool(name="stage", bufs=3) as stpool,
            tc.tile_pool(name="ps_s", bufs=2, space="PSUM") as ps_s,
            tc.tile_pool(name="ps_o", bufs=1, space="PSUM") as ps_o,
        ):
            # ---------- constants / weights into SBUF ----------
            wq_sb = cpool.tile([128, D], BF16)
            nc.sync.dma_start(wq_sb[:].rearrange("p (c m) -> p c m", c=4),
                              wqT[:, :].rearrange("(c p) m -> p c m", p=128))
            wk_sb = cpool.tile([128, D], BF16)
            nc.sync.dma_start(wk_sb[:].rearrange("p (c m) -> p c m", c=4),
                              wkT[:, :].rearrange("(c p) m -> p c m", p=128))
            wv_sb = cpool.tile([128, 4 * 130], BF16)
            nc.sync.dma_start(wv_sb[:].rearrange("p (c m) -> p c m", c=4),
                              wvT[:, :].rearrange("(c p) m -> p c m", p=128))
            wo0_sb = cpool.tile([HD, D], BF16)
            nc.sync.dma_start(wo0_sb[:], wo0[:, :])
            wo1_sb = cpool.tile([HD, D], BF16)
            nc.sync.dma_start(wo1_sb[:], wo1[:, :])
            bq_sb = cpool.tile([R, 1], F32)
            nc.sync.dma_start(bq_sb[:], bq[:, :])
            bk_sb = cpool.tile([R, 1], F32)
            nc.sync.dma_start(bk_sb[:], bk[:, :])
            bvb_sb = cpool.tile([128, 130], F32)
            nc.sync.dma_start(bvb_sb[:], bvb[:, :])
            bob4_sb = cpool.tile([128, D], F32)
            nc.sync.dma_start(bob4_sb[:], bob4[:, :])
            mask_sb = cpool.tile([128, 4 * QW], BF16)
            nc.sync.dma_start(mask_sb[:], maskc[:, :])
            ones_sb = cpool.tile([128, HD], F32)
            nc.vector.memset(ones_sb[:], 1.0)

            xt = []
            for c in range(4):
                t_ = xpool.tile([128, S], BF16, tag=f"xt{c}")
                nc.sync.dma_start(t_[:], xT[128 * c:128 * (c + 1), :])
                xt.append(t_)

            # ---------- Q^T / K^T projections: [128 rows, S] ----------
            qT = xpool.tile([128, S], BF16, tag="qT")
            kT = xpool.tile([128, S], BF16, tag="kT")
            for w_sb, b_sb, dst in ((wq_sb, bq_sb, qT), (wk_sb, bk_sb, kT)):
                for nt in range(NT):
                    ps = ps_s.tile([128, QW], F32, tag="s")
                    for c in range(4):
                        nc.tensor.matmul(
                            ps[:],
                            w_sb[:, 128 * c:128 * (c + 1)],
                            xt[c][:, QW * nt:QW * (nt + 1)],
                            start=(c == 0), stop=(c == 3),
                        )
                    nc.vector.tensor_scalar_add(
                        dst[:, QW * nt:QW * (nt + 1)], ps[:], b_sb[:])

            # ---------- V natural (augmented with ones col per head) ----------
            # vaug[:, 130*tb : 130*tb+130] = [V_h0 | 1 | V_h1 | 1] for token block tb
            vaug = xpool.tile([128, 32 * 130], BF16, tag="vaug")
            for tb in range(32):
                ps = ps_s.tile([128, QW], F32, tag="s")
                for c in range(4):
                    nc.tensor.matmul(
                        ps[:, 0:130],
                        xt[c][:, 128 * tb:128 * (tb + 1)],
                        wv_sb[:, 130 * c:130 * (c + 1)],
                        start=(c == 0), stop=(c == 3),
                    )
                nc.vector.tensor_add(
                    vaug[:, 130 * tb:130 * (tb + 1)], ps[:, 0:130], bvb_sb[:]
                )

            # ---------- attention over q tiles ----------
            for t in range(NT):
                nj = 4 * t + 4          # causal: k blocks 0 .. 4t+3
                o0 = ps_o.tile([65, QW], F32, tag="o0")
                o1 = ps_o.tile([65, QW], F32, tag="o1")
                for j in range(nj):
                    # causal: q columns < q0 are fully masked for this k block
                    q0 = max(0, 128 * (j - 4 * t))
                    live = QW - q0
                    s = ps_s.tile([128, 2 * QW], F32, tag="s")
                    for h in (0, 1):
                        nc.tensor.matmul(
                            s[:, QW * h + q0:QW * (h + 1)],
                            kT[64 * h:64 * (h + 1), 128 * j:128 * (j + 1)],
                            qT[64 * h:64 * (h + 1), QW * t + q0:QW * (t + 1)],
                            start=True, stop=True,
                        )
                    p = ppool.tile([128, 2 * QW], BF16, tag="p")
                    if q0 == 0:
                        nc.scalar.activation(p[:], s[:], AF.Exp, bias=0.0, scale=0.125)
                    else:
                        sv = s[:].rearrange("k (h q) -> k h q", h=2)[:, :, q0:QW]
                        pv = p[:].rearrange("k (h q) -> k h q", h=2)[:, :, q0:QW]
                        nc.scalar.activation(pv, sv, AF.Exp, bias=0.0, scale=0.125)
                    if j >= 4 * t:  # diagonal 128-col boundary: 0/1 mask (r=0 tile)
                        for h in (0, 1):
                            nc.vector.tensor_mul(
                                p[:, QW * h + q0:QW * h + q0 + 128],
                                p[:, QW * h + q0:QW * h + q0 + 128],
                                mask_sb[:, 0:128],
                            )
                    for h, oo in ((0, o0), (1, o1)):
                        nc.tensor.matmul(
                            oo[:, q0:QW],
                            vaug[:, 130 * j + 65 * h:130 * j + 65 * (h + 1)],
                            p[:, QW * h + q0:QW * (h + 1)],
                            start=(j == 0), stop=(j == nj - 1),
                        )

                # ---- softmax denominators (row 64 of o0/o1), normalize, O-proj
                # free the o accumulators fast: copy O^T (rows 0..63) to SBUF
                # and the l rows (row 64) to an SBUF staging row; the O-proj
                # only depends on the copies, not on the reciprocal path.
                oc0 = spool.tile([HD, QW], BF16, tag="oc0")
                oc1 = spool.tile([HD, QW], BF16, tag="oc1")
                nc.vector.tensor_copy(oc0[:], o0[0:64, :])
                nc.vector.tensor_copy(oc1[:], o1[0:64, :])
                lrow = spool.tile([128, 2 * QW], F32, tag="lrow")
                nc.vector.tensor_copy(lrow[64:65, 0:QW], o0[64:65, :])
                nc.vector.tensor_copy(lrow[64:65, QW:2 * QW], o1[64:65, :])

                # l -> natural per-token-partition layout [128, tb] via DMA,
                # then reciprocal (base-0 only: reciprocal_approx_fast quirk)
                l_nat = spool.tile([128, 8], F32, tag="l_nat")
                nc.sync.dma_start(ldram[t][:].rearrange("(a q) -> a q", a=1),
                                  lrow[64:65, :])
                for h in range(2):
                    nc.sync.dma_start(
                        l_nat[:, 4 * h:4 * (h + 1)],
                        ldram[t][QW * h:QW * (h + 1)].rearrange(
                            "(tb p) -> p tb", p=128),
                    )
                linv_nat = spool.tile([128, 8], F32, tag="linv_nat")
                nc.vector.reciprocal_approx_fast(linv_nat[:], l_nat[:])

                for tb in range(4):
                    po0 = ps_o.tile([128, QW], F32, tag="po0")
                    po1 = ps_o.tile([128, QW], F32, tag="po1")
                    nc.tensor.matmul(po0[:], oc0[:, 128 * tb:128 * (tb + 1)],
                                     wo0_sb[:], start=True, stop=True)
                    nc.tensor.matmul(po1[:], oc1[:, 128 * tb:128 * (tb + 1)],
                                     wo1_sb[:], start=True, stop=True)
                    tmp = stpool.tile([128, QW], F32, tag="tmp")
                    nc.vector.scalar_tensor_tensor(
                        tmp[:], po0[:], linv_nat[:, tb:tb + 1], bob4_sb[:],
                        mybir.AluOpType.mult, mybir.AluOpType.add)
                    st = stpool.tile([128, QW], BF16, tag="st")
                    nc.vector.scalar_tensor_tensor(
                        st[:], po1[:], linv_nat[:, 4 + tb:5 + tb], tmp[:],
                        mybir.AluOpType.mult, mybir.AluOpType.add)
                    nc.sync.dma_start(
                        parts[t][128 * tb:128 * (tb + 1), :], st[:]
                    )
                # chunked ReduceScatter: overlaps with later q tiles
                nc.gpsimd.collective_compute(
                    "ReduceScatter",
                    mybir.AluOpType.add,
                    replica_groups=[[0, 1, 2, 3], [4, 5, 6, 7]],
                    ins=[parts[t][:, :]],
                    outs=[rss[t][:, :]],
                )

            # ---------- gather chunks, cast to f32, write output ----------
            for t in range(NT):
                g = stpool.tile([128, D], BF16, tag="g")
                nc.sync.dma_start(g[:], rss[t][:, :])
                gf = stpool.tile([128, D], F32, tag="gf")
                nc.vector.tensor_copy(gf[:], g[:])
                nc.sync.dma_start(out_ext[128 * t:128 * (t + 1), :], gf[:])

    nc.finalize()
    return nc


def _make_in_maps(x, Wqkv, bqkv, Wo, bo):
    # causal 0/1 multiplicative masks for the 4 diagonal sub-block offsets:
    # keep (p, o) where o >= 128*r + p  (k = 128*(4t+r)+p, q = 512*t+o)
    p_idx = np.arange(128)[:, None]
    o_idx = np.arange(QW)[None, :]
    maskc = np.concatenate(
        [(o_idx >= 128 * r + p_idx).astype(np.float32) for r in range(4)], axis=1
    ).astype(BF16_NP)

    in_maps = []
    for core in range(NCORES):
        b = core // 4
        g = core % 4
        rows = slice(128 * g, 128 * (g + 1))
        wq = Wqkv[0:D][rows]            # [128, 512]
        wk = Wqkv[D:2 * D][rows]
        wv = Wqkv[2 * D:3 * D][rows]
        wvT = np.zeros((D, 130), dtype=np.float32)
        wvT[:, 0:64] = wv[0:64].T
        wvT[:, 65:129] = wv[64:128].T
        bvb = np.zeros((128, 130), dtype=np.float32)
        bvb[:, 0:64] = bqkv[2 * D:3 * D][rows][0:64][None, :]
        bvb[:, 64] = 1.0
        bvb[:, 65:129] = bqkv[2 * D:3 * D][rows][64:128][None, :]
        bvb[:, 129] = 1.0
        in_maps.append({
            "xT": np.ascontiguousarray(x[b].T).astype(BF16_NP),
            "wqT": np.ascontiguousarray(wq.T).astype(BF16_NP),
            "wkT": np.ascontiguousarray(wk.T).astype(BF16_NP),
            "wvT": wvT.astype(BF16_NP),
            "bq": np.ascontiguousarray(bqkv[0:D][rows][:, None]).astype(np.float32),
            "bk": np.ascontiguousarray(bqkv[D:2 * D][rows][:, None]).astype(np.float32),
            "bvb": bvb,
            "wo0": np.ascontiguousarray(Wo[:, 128 * g:128 * g + 64].T).astype(BF16_NP),
            "wo1": np.ascontiguousarray(Wo[:, 128 * g + 64:128 * (g + 1)].T).astype(BF16_NP),
            "bob4": np.tile((bo / 4.0).astype(np.float32)[None, :], (128, 1)),
            "maskc": maskc,
        })
    return in_maps


def run(x, Wqkv, bqkv, Wo, bo, trace=False):
    if "nc" not in _CACHE:
        _CACHE["nc"] = _build_nc()
    nc = _CACHE["nc"]
    in_maps = _make_in_maps(x, Wqkv, bqkv, Wo, bo)
    res = run_bass_kernel_spmd(nc, in_maps, core_ids=list(range(NCORES)), trace=trace)
    out = np.empty((B, S, D), dtype=np.float32)
    for core in range(NCORES):
        b = core // 4
        r = core % 4
        o = res.results[core]["out"]
        # chunked ReduceScatter: rank r holds rows 512t+128r..+128 per q tile t
        for t in range(NT):
            out[b, QW * t + 128 * r:QW * t + 128 * (r + 1), :] = o[128 * t:128 * (t + 1)]
    return out, res


def kernel(x, Wqkv, bqkv, Wo, bo):
    out, _ = run(np.asarray(x, dtype=np.float32), np.asarray(Wqkv, dtype=np.float32),
                 np.asarray(bqkv, dtype=np.float32), np.asarray(Wo, dtype=np.float32),
                 np.asarray(bo, dtype=np.float32))
    return out
